# revision 1
# baseline (speedup 1.0000x reference)
"""Trainium2 Bass kernel for nn_CWDiscriminator (per-class 3-layer MLP).

reference:
    x = inputs.transpose(0, 2, 1)            # (B, C, F)
    h = relu(einsum('bcf,cfg->bcg', x, W1) + b1)
    h = relu(einsum('bcf,cfg->bcg', h, W2) + b2)
    out = einsum('bcf,cf->bc', h, W3) + b3   # (B, C)

B=16384, F=256, C=19. Data-parallel over B across 8 NeuronCores
(B_loc = 2048 per core). Per core, per class c:
  - inputs arrive as (B_loc, F*C) bf16 (host-cast); PE transpose-mode
    converts the f-strided slices into X.T tiles (f on partitions).
  - GEMM1 (bf16): H1.T = W1[c].T @ X.T  -> PSUM, evicted by ACT with
    fused bias+ReLU to fp32r.
  - GEMM2 (fp32r): H2.T = W2[c].T @ H1.T -> PSUM, evicted with
    bias+ReLU to fp32r (ACT/DVE split).
  - GEMM3 (fp32r): lhsT = W3 masked to column c (128, 19); all classes
    accumulate into one shared PSUM (19, b) region, so the final
    eviction is one op per half instead of per class.
Output per core is (C, B_loc) fp32; host transposes and adds b3.
"""

import sys
import types

import numpy as np
import ml_dtypes

B, F, C = 16384, 256, 19
NCORES = 8
B_LOC = B // NCORES          # 2048
SECTIONS = [512, 512, 512, 512]  # batch columns per PSUM-accum round
assert sum(SECTIONS) == 2048
NCHUNK = 512                 # matmul moving free dim (one fp32 PSUM bank)
FC = F * C                   # 4864

BF16 = ml_dtypes.bfloat16


# ---------------------------------------------------------------------------
# axon environment shims (NTFF profile hook + artifact upload stub) and the
# one-wait-per-instruction legalizer this walrus build requires.
# ---------------------------------------------------------------------------

def _setup_axon_env():
    if 'antenv.axon_hooks' not in sys.modules:
        mod = types.ModuleType('antenv.axon_hooks')
        mod._hook = None
        mod.set_axon_ntff_profile_hook = lambda h: setattr(mod, '_hook', h)
        mod.get_axon_ntff_profile_hook = lambda: mod._hook
        sys.modules['antenv.axon_hooks'] = mod
        try:
            import antenv
            antenv.axon_hooks = mod
        except ImportError:
            pass
        try:
            from trn_agent_boot.trn_boot import _ntff_profile_via_ctypes
            mod._hook = _ntff_profile_via_ctypes('/opt/axon/libaxon_pjrt.so')
        except Exception:
            pass
    import concourse.bass_utils as bu
    bu.upload_artifacts = lambda tmpdir: 'file://' + str(tmpdir)


def _legalize_waits(nc):
    """walrus accepts at most ONE sync wait per engine instruction (2 for
    EventSemaphore). Split extras onto preceding same-engine NoOps."""
    import concourse.mybir as mybir
    n_split = 0
    for fn in nc.m.functions:
        for bb in fn.blocks:
            insts = bb.instructions
            out = []
            for inst in insts:
                si = inst.sync_info
                ow = list(si.on_wait) if si is not None and si.on_wait else []
                cap = 2 if inst.opcode == "EventSemaphore" else 1
                if len(ow) > cap:
                    keep = ow[-cap:]
                    for k, w in enumerate(ow[:-cap]):
                        nop = mybir.InstNoOp(
                            name=f"{inst.name}-wsplit{k}",
                            engine=inst.engine,
                            ins=[],
                            outs=[],
                            sync_info=mybir.SyncInfo(on_wait=[w], on_update=[]),
                        )
                        out.append(nop)
                        n_split += 1
                    inst.sync_info = mybir.SyncInfo(
                        on_wait=keep,
                        on_update=list(si.on_update) if si.on_update else [],
                    )
                out.append(inst)
            insts[:] = out
    return n_split


# ---------------------------------------------------------------------------
# device program
# ---------------------------------------------------------------------------

_CACHE = {}
last_results = None  # BassKernelResults of the most recent run (for test.py)


def _build_program():
    from contextlib import ExitStack
    import concourse.bass as bass
    import concourse.mybir as mybir
    import concourse.tile as tile

    F32 = mybir.dt.float32
    F32R = mybir.dt.float32r
    B16 = mybir.dt.bfloat16

    nc = bass.Bass()

    # xt: host-pretransposed input, [p, c, k, b] = x[b, 128k+p, c], bf16
    xtd = nc.declare_dram_parameter("xtd", [128, C, 2, B_LOC], B16,
                                    isOutput=False)
    w1t = nc.declare_dram_parameter("w1t", [128, C, 2, 2, 128], B16,
                                    isOutput=False)
    w2t = nc.declare_dram_parameter("w2t", [128, C * 2 * 2 * 128], B16,
                                    isOutput=False)
    w3m = nc.declare_dram_parameter("w3m", [128, C * 2 * C], F32,
                                    isOutput=False)
    b1s = nc.declare_dram_parameter("b1s", [128, C, 2], F32, isOutput=False)
    b2s = nc.declare_dram_parameter("b2s", [128, C, 2], F32, isOutput=False)
    out = nc.declare_dram_parameter("out", [C, B_LOC], F32, isOutput=True)

    NSEC = len(SECTIONS)
    SEC = SECTIONS[0]

    with ExitStack() as ctx:
        tc = ctx.enter_context(tile.TileContext(nc))

        consts = ctx.enter_context(tc.tile_pool(name="consts", bufs=1))
        wtmp_pool = ctx.enter_context(tc.tile_pool(name="wtmp", bufs=1))
        xt_pool = ctx.enter_context(tc.tile_pool(name="xt", bufs=2))
        h1_pool = ctx.enter_context(tc.tile_pool(name="h1p", bufs=2))
        h2_pool = ctx.enter_context(tc.tile_pool(name="h2p", bufs=2))
        out_pool = ctx.enter_context(tc.tile_pool(name="outp", bufs=2))

        ps_g = ctx.enter_context(
            tc.tile_pool(name="ps_g", bufs=6, space="PSUM"))
        ps_3 = ctx.enter_context(
            tc.tile_pool(name="ps_3", bufs=2, space="PSUM"))

        # ---- X.T section slabs stream on the sync ring, self-paced by
        # the xt pool slots; everything else rides the scalar ring.
        # slab 0 loads immediately (split by class range so L1(c=0) can
        # start early); later slabs are emitted inside the previous
        # section's pipeline, gated on its progress, so their DMA doesn't
        # steal HBM bandwidth from the weight loads at startup.
        xts0 = xt_pool.tile([128, C, 2, SEC], B16, tag="xt")
        for c0 in range(0, C, 5):
            c1 = min(c0 + 5, C)
            nc.sync.dma_start(
                xts0[:, c0:c1], xtd[:, c0:c1, :, 0:SEC])
        slabs = [xts0]

        # Weight loads interleaved in class-consumption order: the class-c
        # pipeline needs w1[c] first, w2[c] two iterations later.
        w1sb = consts.tile([128, C, 2, 2, 128], B16)
        w2sb = consts.tile([128, C * 2 * 2 * 128], B16)
        w3sb = consts.tile([128, C * 2 * C], F32R)
        b1sb = consts.tile([128, C, 2], F32)
        b2sb = consts.tile([128, C, 2], F32)
        NW2 = C * 2 * 2 * 128  # 9728

        nc.scalar.dma_start(w1sb[:, 0:5], w1t[:, 0:5])
        nc.scalar.dma_start(b1sb[:], b1s[:])
        nc.scalar.dma_start(b2sb[:], b2s[:])
        nc.scalar.dma_start(w2sb[:, 0:NW2 // 4], w2t[:, 0:NW2 // 4])
        # W3 must be *rounded to fp32r* by a compute op before fp32r
        # matmuls consume it; DMA fp32 then convert on DVE.
        w3tmp = wtmp_pool.tile([128, C * 2 * C], F32, tag="w2tmp")
        nc.scalar.dma_start(w3tmp[:], w3m[:])
        nc.vector.tensor_copy(w3sb[:], w3tmp[:])
        nc.scalar.dma_start(w1sb[:, 5:12], w1t[:, 5:12])
        nc.scalar.dma_start(
            w2sb[:, NW2 // 4:NW2 // 2], w2t[:, NW2 // 4:NW2 // 2])
        nc.scalar.dma_start(w1sb[:, 12:C], w1t[:, 12:C])
        nc.scalar.dma_start(w2sb[:, NW2 // 2:], w2t[:, NW2 // 2:])

        # PE warm-up burst: dummy matmuls while DMA fills SBUF, so the
        # HAM clock-gate reaches 8/8 before the first real GEMM issues.
        wu_l = consts.tile([128, 128], B16)
        nc.gpsimd.memset(wu_l[:], 0.0)
        wu_r = consts.tile([128, 512], B16)
        nc.gpsimd.memset(wu_r[:], 0.0)
        wu_ps = ps_3.tile([128, 512], mybir.dt.float32, tag="ps3")
        for i in range(10):
            nc.tensor.matmul(wu_ps[:], wu_l[:], wu_r[:],
                             start=True, stop=True)

        w1v = w1sb[:]
        w2v = w2sb[:].rearrange("p (c k m j) -> p c k m j", c=C, k=2, m=2)
        w3v = w3sb[:].rearrange("p (c k q) -> p c k q", c=C, k=2)

        for h in range(1, NSEC):
            xts = xt_pool.tile([128, C, 2, SEC], B16, tag="xt",
                               name=f"xts{h}")
            nc.sync.dma_start(xts[:], xtd[:, :, :, h * SEC:(h + 1) * SEC])
            slabs.append(xts)

        for h in range(NSEC):
            xtv = slabs[h][:]
            sec0 = h * SEC
            ps3 = ps_3.tile([C, SEC], mybir.dt.float32, tag="ps3")
            h1_t = [None, None]
            h2_t = [None, None]
            for cc in range(C + 2):
                if cc < C:
                    c = cc
                    h1 = h1_pool.tile([128, 2, SEC], B16, tag="h1")
                    h1_t[c % 2] = h1
                    for m in range(2):
                        pg = ps_g.tile([128, SEC], mybir.dt.float32,
                                       tag="pg")
                        for k in range(2):
                            nc.tensor.matmul(
                                pg[:], w1v[:, c, k, m, :],
                                xtv[:, c, k, :],
                                start=(k == 0), stop=(k == 1))
                        nc.scalar.activation(
                            h1[:, m, :], pg[:],
                            mybir.ActivationFunctionType.Relu,
                            bias=b1sb[:, c, m:m+1])
                if 1 <= cc <= C:
                    c = cc - 1
                    h1 = h1_t[c % 2]
                    h2 = h2_pool.tile([128, 2, SEC], F32R, tag="h2")
                    h2_t[c % 2] = h2
                    for m in range(2):
                        pg = ps_g.tile([128, SEC], mybir.dt.float32,
                                       tag="pg")
                        for k in range(2):
                            nc.tensor.matmul(
                                pg[:], w2v[:, c, k, m, :],
                                h1[:, k, :],
                                start=(k == 0), stop=(k == 1))
                        nc.vector.tensor_scalar(
                            h2[:, m, :], pg[:],
                            b2sb[:, c, m:m+1], 0.0,
                            mybir.AluOpType.add, mybir.AluOpType.max)
                if cc >= 2:
                    c = cc - 2
                    h2 = h2_t[c % 2]
                    for k in range(2):
                        nc.tensor.matmul(
                            ps3[:], w3v[:, c, k, :], h2[:, k, :],
                            start=(c == 0 and k == 0),
                            stop=(c == C - 1 and k == 1))

            out_sb = out_pool.tile([C, SEC], F32, tag="osb")
            nc.vector.tensor_copy(out_sb[:], ps3[:])
            nc.scalar.dma_start(out[:, sec0:sec0 + SEC], out_sb[:])

    _legalize_waits(nc)
    return nc


def _get_program():
    if 'nc' not in _CACHE:
        _setup_axon_env()
        _CACHE['nc'] = _build_program()
    return _CACHE['nc']


# ---------------------------------------------------------------------------
# host wrapper
# ---------------------------------------------------------------------------

def kernel(inputs, W1, b1, W2, b2, W3, b3):
    global last_results
    from concourse.bass_utils import run_bass_kernel_spmd

    nc = _get_program()

    inputs = np.asarray(inputs)
    W1 = np.asarray(W1, dtype=np.float32)
    b1 = np.asarray(b1, dtype=np.float32)
    W2 = np.asarray(W2, dtype=np.float32)
    b2 = np.asarray(b2, dtype=np.float32)
    W3 = np.asarray(W3, dtype=np.float32)
    b3 = np.asarray(b3, dtype=np.float32)

    # host-side layout prep for the shard: [p, c, k, b] = x[b, 128k+p, c]
    xbf = np.asarray(inputs).reshape(B, 2, 128, C).astype(BF16)
    xtd_full = np.ascontiguousarray(xbf.transpose(2, 3, 1, 0))

    # lhsT tiles: w1t[p, c, k, m, j] = W1[c, 128k+p, 128m+j]
    w1t = np.ascontiguousarray(
        W1.reshape(C, 2, 128, 2, 128).transpose(2, 0, 1, 3, 4)).astype(BF16)
    w2t = np.ascontiguousarray(
        W2.reshape(C, 2, 128, 2, 128).transpose(2, 0, 1, 3, 4)
    ).reshape(128, C * 2 * 2 * 128).astype(BF16)
    # w3m[p, c, k, c'] = (c'==c) * W3[c, 128k+p]
    w3m = np.zeros((128, C, 2, C), dtype=np.float32)
    for c in range(C):
        w3m[:, c, 0, c] = W3[c, :128]
        w3m[:, c, 1, c] = W3[c, 128:]
    w3m = w3m.reshape(128, C * 2 * C)
    # b1s[p, c, m] = b1[c, 128m+p]
    b1s = np.ascontiguousarray(
        b1.reshape(C, 2, 128).transpose(2, 0, 1)).astype(np.float32)
    b2s = np.ascontiguousarray(
        b2.reshape(C, 2, 128).transpose(2, 0, 1)).astype(np.float32)

    core_ids = list(range(NCORES))
    in_maps = []
    for i in core_ids:
        in_maps.append({
            "xtd": np.ascontiguousarray(
                xtd_full[:, :, :, i * B_LOC:(i + 1) * B_LOC]),
            "w1t": w1t, "w2t": w2t, "w3m": w3m, "b1s": b1s, "b2s": b2s,
        })

    import os
    trace = bool(os.environ.get("BASS_TRACE"))
    res = run_bass_kernel_spmd(nc, in_maps, core_ids, trace=trace)
    last_results = res

    out_full = np.empty((B, C), dtype=np.float32)
    for i in core_ids:
        out_full[i * B_LOC:(i + 1) * B_LOC] = res.results[i]["out"].T
    out_full += b3[None, :]
    return out_full



# revision 2
# speedup vs baseline: 1.0669x; 1.0669x over previous
"""Trainium2 Bass kernel for nn_CWDiscriminator (per-class 3-layer MLP).

reference:
    x = inputs.transpose(0, 2, 1)            # (B, C, F)
    h = relu(einsum('bcf,cfg->bcg', x, W1) + b1)
    h = relu(einsum('bcf,cfg->bcg', h, W2) + b2)
    out = einsum('bcf,cf->bc', h, W3) + b3   # (B, C)

B=16384, F=256, C=19. Data-parallel over B across 8 NeuronCores
(B_loc = 2048 per core). Per core, per 512-column batch section:
  - GEMM1 (bf16): H1.T = W1[c].T @ X.T -> PSUM, ACT evicts with fused
    bias+ReLU to bf16.
  - GEMM2 (bf16): H2.T = W2[c].T @ H1.T -> PSUM, DVE evicts with
    bias+ReLU to bf16; all 19 h2 tiles stay resident for the section.
  - GEMM3: col-tiled. Classes map to the four 32-partition column
    groups of the PE array (strip j = c//5, local row i = c%5,
    psum partition 32j+i). Four classes' matmuls run concurrently via
    tile_position=(0,32j), all accumulating into one (128,512) PSUM
    bank. A zero matmul clears has_written for the whole bank first so
    the strip matmuls never need start=True (whose bank-wide clear
    would race between concurrent strips).
Output per core is (C, B_LOC) fp32; host transposes and adds b3.
"""

import sys
import types

import numpy as np
import ml_dtypes

B, F, C = 16384, 256, 19
NCORES = 8
B_LOC = B // NCORES          # 2048
SECTIONS = [512, 512, 512, 512]  # batch columns per section
assert sum(SECTIONS) == B_LOC
NSEC = len(SECTIONS)
SEC = SECTIONS[0]

BF16 = ml_dtypes.bfloat16

# class c -> column strip j = c // 5 (partitions 32j..), local row i = c % 5
NSTRIP = 4
STRIP_N = [5, 5, 5, 4]       # classes per strip


# ---------------------------------------------------------------------------
# axon environment shims (NTFF profile hook + artifact upload stub) and the
# one-wait-per-instruction legalizer this walrus build requires.
# ---------------------------------------------------------------------------

def _setup_axon_env():
    if 'antenv.axon_hooks' not in sys.modules:
        mod = types.ModuleType('antenv.axon_hooks')
        mod._hook = None
        mod.set_axon_ntff_profile_hook = lambda h: setattr(mod, '_hook', h)
        mod.get_axon_ntff_profile_hook = lambda: mod._hook
        sys.modules['antenv.axon_hooks'] = mod
        try:
            import antenv
            antenv.axon_hooks = mod
        except ImportError:
            pass
        try:
            from trn_agent_boot.trn_boot import _ntff_profile_via_ctypes
            mod._hook = _ntff_profile_via_ctypes('/opt/axon/libaxon_pjrt.so')
        except Exception:
            pass
    import concourse.bass_utils as bu
    bu.upload_artifacts = lambda tmpdir: 'file://' + str(tmpdir)


def _legalize_waits(nc):
    """walrus accepts at most ONE sync wait per engine instruction (2 for
    EventSemaphore). Split extras onto preceding same-engine NoOps."""
    import concourse.mybir as mybir
    n_split = 0
    for fn in nc.m.functions:
        for bb in fn.blocks:
            insts = bb.instructions
            out = []
            for inst in insts:
                si = inst.sync_info
                ow = list(si.on_wait) if si is not None and si.on_wait else []
                cap = 2 if inst.opcode == "EventSemaphore" else 1
                if len(ow) > cap:
                    keep = ow[-cap:]
                    for k, w in enumerate(ow[:-cap]):
                        nop = mybir.InstNoOp(
                            name=f"{inst.name}-wsplit{k}",
                            engine=inst.engine,
                            ins=[],
                            outs=[],
                            sync_info=mybir.SyncInfo(on_wait=[w], on_update=[]),
                        )
                        out.append(nop)
                        n_split += 1
                    inst.sync_info = mybir.SyncInfo(
                        on_wait=keep,
                        on_update=list(si.on_update) if si.on_update else [],
                    )
                out.append(inst)
            insts[:] = out
    return n_split


# ---------------------------------------------------------------------------
# device program
# ---------------------------------------------------------------------------

_CACHE = {}
last_results = None  # BassKernelResults of the most recent run (for test.py)

# class-range chunks for the startup loads (finer first so class 0's
# pipeline starts as early as possible)
W_CHUNKS = [(0, 2), (2, 6), (6, 12), (12, C)]


def _build_program():
    from contextlib import ExitStack
    import concourse.bass as bass
    import concourse.mybir as mybir
    import concourse.tile as tile

    F32 = mybir.dt.float32
    B16 = mybir.dt.bfloat16

    nc = bass.Bass()

    # xt: host-pretransposed input, [h, p, c, k, b] = x[512h+b, 128k+p, c]
    xtd = nc.declare_dram_parameter("xtd", [NSEC, 128, C, 2, SEC], B16,
                                    isOutput=False)
    w1t = nc.declare_dram_parameter("w1t", [128, C, 2, 2, 128], B16,
                                    isOutput=False)
    w2t = nc.declare_dram_parameter("w2t", [128, C, 2, 2, 128], B16,
                                    isOutput=False)
    # w3p[p, c, k, i] = (i == c%5) * W3[c, 128k+p]
    w3p = nc.declare_dram_parameter("w3p", [128, C, 2, 5], B16,
                                    isOutput=False)
    b1s = nc.declare_dram_parameter("b1s", [128, C, 2], F32, isOutput=False)
    b2s = nc.declare_dram_parameter("b2s", [128, C, 2], F32, isOutput=False)
    out = nc.declare_dram_parameter("out", [C, B_LOC], F32, isOutput=True)

    with ExitStack() as ctx:
        tc = ctx.enter_context(tile.TileContext(nc))

        consts = ctx.enter_context(tc.tile_pool(name="consts", bufs=1))
        xt_pool = ctx.enter_context(tc.tile_pool(name="xt", bufs=2))
        h1_pool = ctx.enter_context(tc.tile_pool(name="h1p", bufs=2))
        h2_pool = ctx.enter_context(tc.tile_pool(name="h2p", bufs=C))
        out_pool = ctx.enter_context(tc.tile_pool(name="outp", bufs=2))

        ps_g = ctx.enter_context(
            tc.tile_pool(name="ps_g", bufs=6, space="PSUM"))
        ps_3 = ctx.enter_context(
            tc.tile_pool(name="ps_3", bufs=2, space="PSUM"))

        # ---- SBUF tiles for weights/biases
        w1sb = consts.tile([128, C, 2, 2, 128], B16)
        w2sb = consts.tile([128, C, 2, 2, 128], B16)
        w3sb = consts.tile([128, C, 2, 5], B16)
        b1sb = consts.tile([128, C, 2], F32)
        b2sb = consts.tile([128, C, 2], F32)

        # ---- X.T section slabs on the sync ring; weights on the scalar
        # ring. Both rings interleave class-consumption order so class 0
        # can start after ~0.8 MB instead of the full ~10 MB.
        xts0 = xt_pool.tile([128, C, 2, SEC], B16, tag="xt")
        nc.sync.dma_start(xts0[:, 0:2], xtd[0, :, 0:2])
        nc.scalar.dma_start(w1sb[:, 0:2], w1t[:, 0:2])
        nc.scalar.dma_start(w2sb[:, 0:2], w2t[:, 0:2])
        nc.scalar.dma_start(b1sb[:], b1s[:])
        nc.scalar.dma_start(b2sb[:], b2s[:])
        for c0, c1 in W_CHUNKS[1:]:
            nc.sync.dma_start(xts0[:, c0:c1], xtd[0, :, c0:c1])
            nc.scalar.dma_start(w1sb[:, c0:c1], w1t[:, c0:c1])
            nc.scalar.dma_start(w2sb[:, c0:c1], w2t[:, c0:c1])
        nc.scalar.dma_start(w3sb[:], w3p[:])
        slabs = [xts0]
        for h in range(1, NSEC):
            xts = xt_pool.tile([128, C, 2, SEC], B16, tag="xt",
                               name=f"xts{h}")
            nc.sync.dma_start(xts[:], xtd[h])
            slabs.append(xts)

        # PE warm-up burst while the first DMA chunks land, so the HAM
        # clock-gate reaches 8/8 before the first real GEMM issues.
        # Memsets on DVE (fast) so the burst starts immediately.
        wu_l = consts.tile([128, 128], B16)
        wu_r = consts.tile([128, 512], B16)
        nc.vector.memset(wu_l[:], 0.0)
        nc.vector.memset(wu_r[:], 0.0)
        wu_ps = ps_3.tile([128, 512], mybir.dt.float32, tag="ps3")
        for i in range(8):
            nc.tensor.matmul(wu_ps[:], wu_l[:], wu_r[:],
                             start=True, stop=True)

        w1v = w1sb[:]
        w2v = w2sb[:]
        w3v = w3sb[:]

        for h in range(NSEC):
            xtv = slabs[h][:]
            sec0 = h * SEC
            ps3 = ps_3.tile([128, SEC], mybir.dt.float32, tag="ps3")
            h1_t = [None, None]
            h2_t = [None] * C
            for cc in range(C + 1):
                if cc < C:
                    c = cc
                    h1 = h1_pool.tile([128, 2, SEC], B16, tag="h1")
                    h1_t[c % 2] = h1
                    for m in range(2):
                        pg = ps_g.tile([128, SEC], mybir.dt.float32,
                                       tag="pg")
                        for k in range(2):
                            nc.tensor.matmul(
                                pg[:], w1v[:, c, k, m, :],
                                xtv[:, c, k, :],
                                start=(k == 0), stop=(k == 1))
                        nc.scalar.activation(
                            h1[:, m, :], pg[:],
                            mybir.ActivationFunctionType.Relu,
                            bias=b1sb[:, c, m:m+1])
                if cc >= 1:
                    c = cc - 1
                    h1 = h1_t[c % 2]
                    h2 = h2_pool.tile([128, 2, SEC], B16, tag="h2")
                    h2_t[c] = h2
                    for m in range(2):
                        pg = ps_g.tile([128, SEC], mybir.dt.float32,
                                       tag="pg")
                        for k in range(2):
                            nc.tensor.matmul(
                                pg[:], w2v[:, c, k, m, :],
                                h1[:, k, :],
                                start=(k == 0), stop=(k == 1))
                        nc.vector.tensor_scalar(
                            h2[:, m, :], pg[:],
                            b2sb[:, c, m:m+1], 0.0,
                            mybir.AluOpType.add, mybir.AluOpType.max)

            # ---- GEMM3, col-tiled: zero-fill matmul sets has_written
            # for the whole bank, then per-round groups of 4 concurrent
            # strip matmuls accumulate (start=False throughout).
            nc.tensor.matmul(ps3[:], wu_l[:], wu_r[:, 0:SEC],
                             start=True, stop=False, skip_group_check=True)
            for i in range(5):
                for k in range(2):
                    for j in range(NSTRIP):
                        c = 5 * j + i
                        if c >= C:
                            continue
                        last = (i == 4 and k == 1 and j == 2)
                        nc.tensor.matmul(
                            ps3[32 * j:32 * j + i + 1, :],
                            w3v[:, c, k, 0:i + 1],
                            h2_t[c][:, k, :],
                            start=False, stop=last,
                            tile_position=(0, 32 * j),
                            skip_group_check=True)

            out_sb = out_pool.tile([128, SEC], F32, tag="osb")
            nc.vector.tensor_copy(out_sb[:], ps3[:])
            for j in range(NSTRIP):
                nc.scalar.dma_start(
                    out[5 * j:5 * j + STRIP_N[j], sec0:sec0 + SEC],
                    out_sb[32 * j:32 * j + STRIP_N[j], :])

    _legalize_waits(nc)
    return nc


def _get_program():
    if 'nc' not in _CACHE:
        _setup_axon_env()
        _CACHE['nc'] = _build_program()
    return _CACHE['nc']


# ---------------------------------------------------------------------------
# host wrapper
# ---------------------------------------------------------------------------

def kernel(inputs, W1, b1, W2, b2, W3, b3):
    global last_results
    from concourse.bass_utils import run_bass_kernel_spmd

    nc = _get_program()

    inputs = np.asarray(inputs)
    W1 = np.asarray(W1, dtype=np.float32)
    b1 = np.asarray(b1, dtype=np.float32)
    W2 = np.asarray(W2, dtype=np.float32)
    b2 = np.asarray(b2, dtype=np.float32)
    W3 = np.asarray(W3, dtype=np.float32)
    b3 = np.asarray(b3, dtype=np.float32)

    # host-side layout prep: xtd[h, p, c, k, b] = x[(section h)b, 128k+p, c]
    xbf = inputs.reshape(B, 2, 128, C).astype(BF16)
    xtd_full = xbf.transpose(2, 3, 1, 0)      # (128, C, 2, B)

    # lhsT tiles: w{1,2}t[p, c, k, m, j] = W[c, 128k+p, 128m+j]
    w1t = np.ascontiguousarray(
        W1.reshape(C, 2, 128, 2, 128).transpose(2, 0, 1, 3, 4)).astype(BF16)
    w2t = np.ascontiguousarray(
        W2.reshape(C, 2, 128, 2, 128).transpose(2, 0, 1, 3, 4)).astype(BF16)
    # w3p[p, c, k, i] = (i == c%5) * W3[c, 128k+p]
    w3p = np.zeros((128, C, 2, 5), dtype=np.float32)
    for c in range(C):
        w3p[:, c, 0, c % 5] = W3[c, :128]
        w3p[:, c, 1, c % 5] = W3[c, 128:]
    w3p = w3p.astype(BF16)
    # b1s[p, c, m] = b1[c, 128m+p]
    b1s = np.ascontiguousarray(
        b1.reshape(C, 2, 128).transpose(2, 0, 1)).astype(np.float32)
    b2s = np.ascontiguousarray(
        b2.reshape(C, 2, 128).transpose(2, 0, 1)).astype(np.float32)

    core_ids = list(range(NCORES))
    in_maps = []
    for i in core_ids:
        xc = xtd_full[:, :, :, i * B_LOC:(i + 1) * B_LOC]
        xc = np.ascontiguousarray(
            xc.reshape(128, C, 2, NSEC, SEC).transpose(3, 0, 1, 2, 4))
        in_maps.append({
            "xtd": xc,
            "w1t": w1t, "w2t": w2t, "w3p": w3p, "b1s": b1s, "b2s": b2s,
        })

    import os
    trace = bool(os.environ.get("BASS_TRACE"))
    res = run_bass_kernel_spmd(nc, in_maps, core_ids, trace=trace)
    last_results = res

    out_full = np.empty((B, C), dtype=np.float32)
    for i in core_ids:
        out_full[i * B_LOC:(i + 1) * B_LOC] = res.results[i]["out"].T
    out_full += b3[None, :]
    return out_full
